# revision 11
# baseline (speedup 1.0000x reference)
"""Trainium2 Bass kernel for DPB (dynamic-position-bias) windowed attention.

Full inputs in, full outputs out. Shards data-parallel over the batch/window
axis across 8 NeuronCores (32 windows per core); all weights and the position
bias table are replicated per core.

Per-core pipeline for each window b (N=256 tokens, C=256 ch, 8 heads, d=32):
  1. qkT = (Wqk x_b)          PE, fp32r, output (o, n) layout
  2. v   = (x_b^T Wv)         PE, output (m, o_v) layout
  3. per head pair: PSUM <- rpb bias (identity-matmul preload), then
     ST[m,n] += k_h^T q_h     K=32 row-packed matmuls (distinct PSUM banks)
  4. P = exp(ST)              ScalarE, FD=1024, PSUM->SBUF (no max-subtract:
                              logits are O(1) so fp32 exp is safe)
  5. out_h^T = v_h^T P, den_h = 1^T P   3-way col-packed matmuls
  6. outT = out_h^T * recip(den)        VectorE (softmax normalize)
  7. final = Wproj outT       PE, written back in (C, N) layout
"""

import os
import sys

sys.path.insert(0, "/opt/trn_rl_repo")

import ml_dtypes
import numpy as np
from contextlib import ExitStack

import concourse.bacc as bacc
import concourse.tile as tile
from concourse import mybir
from concourse.bass_utils import run_bass_kernel_spmd

F32 = mybir.dt.float32
F32R = mybir.dt.float32r
BF16 = mybir.dt.bfloat16
EXP = mybir.ActivationFunctionType.Exp

NUM_HEADS = 8
B, C, H, W = 256, 256, 16, 16
N = H * W            # 256 tokens per window
D = C // NUM_HEADS   # 32
PD = 16              # pos-MLP hidden dim
LN_EPS = 1e-5
NCORES = 8
BPC = int(os.environ.get("KERNEL_BPC", B // NCORES))  # windows per core (32)


# ---------------------------------------------------------------- host math
def _rel_index():
    coords = np.stack(np.meshgrid(np.arange(H), np.arange(W), indexing="ij"))
    cf = coords.reshape(2, -1)
    rel = (cf[:, :, None] - cf[:, None, :]).transpose(1, 2, 0)
    rel = rel + np.array([H - 1, W - 1])
    rel[..., 0] *= 2 * W - 1
    return rel.sum(-1)  # (N, N) int


def _biases():
    bh = np.arange(1 - H, H)
    bw = np.arange(1 - W, W)
    b = np.stack(np.meshgrid(bh, bw, indexing="ij")).reshape(2, -1).T
    return b.astype(np.float32)  # ((2H-1)(2W-1), 2)


def _ln(x, g, b):
    mu = x.mean(-1, keepdims=True)
    var = ((x - mu) ** 2).mean(-1, keepdims=True)
    return (x - mu) / np.sqrt(var + LN_EPS) * g + b


def _pos_mlp(pos_proj_w, pos_proj_b, ln1_g, ln1_b, fc1_w, fc1_b,
             ln2_g, ln2_b, fc2_w, fc2_b, ln3_g, ln3_b, fc3_w, fc3_b):
    p = _biases() @ pos_proj_w.T + pos_proj_b
    p = np.maximum(_ln(p, ln1_g, ln1_b), 0.0) @ fc1_w.T + fc1_b
    p = np.maximum(_ln(p, ln2_g, ln2_b), 0.0) @ fc2_w.T + fc2_b
    p = np.maximum(_ln(p, ln3_g, ln3_b), 0.0) @ fc3_w.T + fc3_b
    return p.astype(np.float32)  # (L, heads)


# ------------------------------------------------------------- bass program
_PROGRAM_CACHE = {}
_LAST_RESULTS = None


def _build_program(has_qkvb, has_projb):
    key = (has_qkvb, has_projb, os.environ.get("KERNEL_DEBUG", "0"))
    if key in _PROGRAM_CACHE:
        return _PROGRAM_CACHE[key]

    nc = bacc.Bacc("TRN2", target_bir_lowering=False, debug=False)

    def din(name, shp, dt=F32R):
        return nc.dram_tensor(name, shp, dt, kind="ExternalInput").ap()

    x_d = din("x", [BPC, C, N])                 # per-core x slice, (b, c, n)
    wqk_d = din("wqk", [2, 128, 512])           # (c-chunk, c_p, o) q|k, q scaled
    wv_d = din("wv", [2, 128, 256])             # (c-chunk, c_p, o_v)
    wp_d = din("wp", [128, 3, 256])             # proj weights, permuted + padded
    rpb_d = din("rpb", [128, NUM_HEADS, 2, 256])  # transposed bias table
    ident_d = din("ident", [128, 128])
    onesw_d = din("onesw", [128, 32], BF16)
    zout_d = din("zout", [128, 3, 256])
    qkvb_d = din("qkvb", [1, 768])
    projb_d = din("projb", [1, 256])
    onesr_d = din("onesr", [1, 256])
    out_d = nc.dram_tensor("out", [BPC, C, N], F32, kind="ExternalOutput").ap()
    DEBUG = bool(int(os.environ.get("KERNEL_DEBUG", "0")))
    if DEBUG:
        dbg_qk = nc.dram_tensor("dbg_qk", [128, 4, 256], F32, kind="ExternalOutput").ap()
        dbg_v = nc.dram_tensor("dbg_v", [128, 2, 256], F32, kind="ExternalOutput").ap()
        dbg_p = nc.dram_tensor("dbg_p", [128, 2, 2, 256], F32, kind="ExternalOutput").ap()
        dbg_outT = nc.dram_tensor("dbg_outT", [128, 3, 256], F32, kind="ExternalOutput").ap()
        dbg_avden = nc.dram_tensor("dbg_avden", [128, 2, 256], F32, kind="ExternalOutput").ap()

    with tile.TileContext(nc) as tc, ExitStack() as ctx:
        consts = ctx.enter_context(tc.tile_pool(name="consts", bufs=1))
        px = ctx.enter_context(tc.tile_pool(name="px", bufs=3))
        pqk = ctx.enter_context(tc.tile_pool(name="pqk", bufs=3))
        pv = ctx.enter_context(tc.tile_pool(name="pv", bufs=3))
        pp = ctx.enter_context(tc.tile_pool(name="pp", bufs=5))
        prec = ctx.enter_context(tc.tile_pool(name="prec", bufs=3))
        pout = ctx.enter_context(tc.tile_pool(name="pout", bufs=3))
        # PSUM: st pair-tiles 2 banks x bufs=2 (4 banks) + shared 1-bank pool
        # bufs=4 (4 banks) = all 8 banks.
        pst = ctx.enter_context(tc.tile_pool(name="pst", bufs=2, space="PSUM"))
        psh = ctx.enter_context(tc.tile_pool(name="psh", bufs=4, space="PSUM"))

        wqk_s = consts.tile([128, 2, 512], F32R)
        nc.sync.dma_start(wqk_s[:], wqk_d.transpose([1, 0, 2]))
        wv_s = consts.tile([128, 2, 256], F32R)
        nc.sync.dma_start(wv_s[:], wv_d.transpose([1, 0, 2]))
        wp_s = consts.tile([128, 3, 256], F32R)
        nc.sync.dma_start(wp_s[:], wp_d[:])
        rpb_s = consts.tile([128, NUM_HEADS, 2, 256], F32R)
        nc.sync.dma_start(rpb_s[:], rpb_d[:])
        ident_s = consts.tile([128, 128], F32R)
        nc.sync.dma_start(ident_s[:], ident_d[:])
        onesw_s = consts.tile([128, 32], BF16)
        nc.sync.dma_start(onesw_s[:], onesw_d[:])
        qkvb_s = consts.tile([1, 768], F32R)
        nc.sync.dma_start(qkvb_s[:], qkvb_d[:])
        projb_s = consts.tile([1, 256], F32R)
        nc.sync.dma_start(projb_s[:], projb_d[:])
        onesr_s = consts.tile([1, 256], F32R)
        nc.sync.dma_start(onesr_s[:], onesr_d[:])

        # persistent outT buffers (alternating per batch parity); partitions
        # beyond the valid head rows stay zero so proj can use K=128 blindly
        outT = [consts.tile([128, 3, 256], F32R, name=f"outT{i}", tag=f"outT{i}")
                for i in range(2)]
        nc.sync.dma_start(outT[0][:], zout_d[:])
        nc.sync.dma_start(outT[1][:], zout_d[:])

        G3 = [(0, 1, 2), (3, 4, 5), (6, 7)]  # col-pack head groups

        for b in range(BPC):
            x_sb = px.tile([128, 2, 256], F32R, tag="x")
            nc.sync.dma_start(x_sb[:], x_d[b].rearrange("(cc p) n -> p cc n", p=128))

            # ---- qk projection: out (o, n), oc tiles 0..3 = q0 q1 k0 k1
            qk_ps = psh.tile([128, 2, 256], F32, tag="bank")
            qk_ps2 = psh.tile([128, 2, 256], F32, tag="bank")
            qk_psl = [qk_ps, qk_ps2]
            for oc in range(4):
                dst = qk_psl[oc // 2][:, oc % 2, :]
                osl = slice(oc * 128, oc * 128 + 128)
                nc.tensor.matmul(dst, wqk_s[:, 0, osl], x_sb[:, 0, :],
                                 start=True, stop=False)
                nc.tensor.matmul(dst, wqk_s[:, 1, osl], x_sb[:, 1, :],
                                 start=False, stop=not has_qkvb)
                if has_qkvb:
                    nc.tensor.matmul(dst, qkvb_s[:, osl], onesr_s[:, :],
                                     start=False, stop=True)
            qk_sb = pqk.tile([128, 4, 256], F32R, tag="qk")
            nc.vector.tensor_copy(qk_sb[:, 0:2, :], qk_ps[:, :, :])
            nc.vector.tensor_copy(qk_sb[:, 2:4, :], qk_ps2[:, :, :])
            if DEBUG and b == 0:
                nc.sync.dma_start(dbg_qk[:], qk_sb.bitcast(F32)[:])

            # ---- v projection: out (m, o_v)
            v_ps = psh.tile([128, 2, 256], F32, tag="bank")
            for mc in range(2):
                msl = slice(mc * 128, mc * 128 + 128)
                nc.tensor.matmul(v_ps[:, mc, :], x_sb[:, 0, msl], wv_s[:, 0, :],
                                 start=True, stop=False)
                nc.tensor.matmul(v_ps[:, mc, :], x_sb[:, 1, msl], wv_s[:, 1, :],
                                 start=False, stop=not has_qkvb)
                if has_qkvb:
                    nc.tensor.matmul(v_ps[:, mc, :], onesr_s[:, 0:128],
                                     qkvb_s[:, 512:768], start=False, stop=True)
            v_sb = pv.tile([128, 2, 256], BF16, tag="v")
            nc.vector.tensor_copy(v_sb[:], v_ps[:])
            if DEBUG and b == 0:
                dbgv_sb = pv.tile([128, 2, 256], F32, tag="dbgv")
                nc.vector.tensor_copy(dbgv_sb[:], v_sb[:])
                nc.sync.dma_start(dbg_v[:], dbgv_sb[:])

            # ---- attention scores + softmax numerator, head pairs
            p_tiles = []
            for pg in range(4):  # heads (2pg, 2pg+1)
                st = pst.tile([128, 2, 2, 256], F32, tag="st")
                for par in range(2):
                    h = 2 * pg + par
                    nc.tensor.matmul(st[:, par, :, :], ident_s[:, :],
                                     rpb_s[:, h, :, :], start=True, stop=False)
                for par in range(2):
                    h = 2 * pg + par
                    hl = h % 4
                    hsl = slice(hl * 32, hl * 32 + 32)
                    qoc = h // 4        # q tile col in qk_sb
                    koc = 2 + h // 4    # k tile col
                    for mc in range(2):
                        nc.tensor.matmul(
                            st[:, par, mc, :],
                            qk_sb[hsl, koc, mc * 128:mc * 128 + 128],
                            qk_sb[hsl, qoc, :],
                            start=False, stop=(mc == 1),
                            tile_position=(hl * 32, 0))
                p_sb = pp.tile([128, 2, 2, 256], BF16, tag="p")
                nc.scalar.activation(p_sb[:, :, :, :], st[:, :, :, :], EXP)
                p_tiles.append(p_sb)
                if DEBUG and b == 0 and pg == 0:
                    dbgp_sb = pp.tile([128, 2, 2, 256], F32, tag="dbgp")
                    nc.vector.tensor_copy(dbgp_sb[:], p_sb[:])
                    nc.sync.dma_start(dbg_p[:], dbgp_sb[:])

            # ---- attn @ v and denominators, 3-way col-packed groups
            for t, heads in enumerate(G3):
                # av and den interleave accumulation groups, so they must sit
                # in different PSUM banks (start=True clears whole-bank bits)
                av_ps = psh.tile([128, 256], F32, tag="bank")
                den_ps = psh.tile([128, 256], F32, tag="bank")
                for j, h in enumerate(heads):
                    jsl = slice(j * 32, j * 32 + 32)
                    for mc in range(2):
                        p_ap = p_tiles[h // 2][:, h % 2, mc, :]
                        nc.tensor.matmul(av_ps[jsl, :],
                                         v_sb[:, mc, h * 32:h * 32 + 32], p_ap,
                                         start=(mc == 0), stop=(mc == 1),
                                         tile_position=(0, j * 32))
                        nc.tensor.matmul(den_ps[jsl, :], onesw_s[:, :], p_ap,
                                         start=(mc == 0), stop=(mc == 1),
                                         tile_position=(0, j * 32))
                if DEBUG and b == 0 and t == 0:
                    dbgav_sb = prec.tile([128, 2, 256], F32, tag="dbgav")
                    nc.vector.tensor_copy(dbgav_sb[:, 0, :], av_ps[:])
                    nc.vector.tensor_copy(dbgav_sb[:, 1, :], den_ps[:])
                    nc.sync.dma_start(dbg_avden[:], dbgav_sb[:])
                nv = 32 * len(heads)
                rec = prec.tile([128, 256], F32, tag="rec")
                nc.vector.reciprocal(rec[0:nv, :], den_ps[0:nv, :])
                nc.vector.tensor_mul(outT[b % 2][0:nv, t, :],
                                     av_ps[0:nv, :], rec[0:nv, :])

            if DEBUG and b == 0:
                nc.sync.dma_start(dbg_outT[:], outT[0].bitcast(F32)[:])
            # ---- output projection: final (o, n)
            f_ps = psh.tile([128, 2, 256], F32, tag="bank")
            for oc in range(2):
                osl = slice(oc * 128, oc * 128 + 128)
                for t in range(3):
                    nc.tensor.matmul(f_ps[:, oc, :], wp_s[:, t, osl],
                                     outT[b % 2][:, t, :],
                                     start=(t == 0),
                                     stop=(t == 2 and not has_projb))
                if has_projb:
                    nc.tensor.matmul(f_ps[:, oc, :], projb_s[:, osl],
                                     onesr_s[:, :], start=False, stop=True)
            out_sb = pout.tile([128, 2, 256], F32, tag="out")
            nc.vector.tensor_copy(out_sb[:], f_ps[:])
            nc.sync.dma_start(out_d[b].rearrange("(oc p) n -> p oc n", p=128),
                              out_sb[:])

    nc.compile()
    _PROGRAM_CACHE[key] = nc
    return nc


# ------------------------------------------------------------------ kernel
def host_prep(inp):
    """Returns (in_maps, has_qkvb, has_projb) for the 8 cores."""
    x = inp["x"].reshape(B, C, N)
    scale = float(D) ** -0.5

    # fold the attention scale into the q columns of qkv_w
    qkv_wT = inp["qkv_w"].T.copy()            # (c, o)
    qkv_wT[:, 0:C] *= scale
    wqk = np.ascontiguousarray(
        qkv_wT[:, 0:512].reshape(2, 128, 512))
    wv = np.ascontiguousarray(
        qkv_wT[:, 512:768].reshape(2, 128, 256))

    # proj weights: permuted into the 3-way col-pack outT layout, zero-padded
    proj_wT = inp["proj_w"].T.copy()          # (c, o)
    wp = np.zeros((128, 3, 256), np.float32)
    G3 = [(0, 1, 2), (3, 4, 5), (6, 7)]
    for t, heads in enumerate(G3):
        for j, h in enumerate(heads):
            wp[j * 32:(j + 1) * 32, t, :] = proj_wT[h * 32:(h + 1) * 32, :]

    # dynamic position bias table via the tiny MLP (host, fp32)
    p_mlp = _pos_mlp(
        inp["pos_proj_w"], inp["pos_proj_b"], inp["ln1_g"], inp["ln1_b"],
        inp["fc1_w"], inp["fc1_b"], inp["ln2_g"], inp["ln2_b"],
        inp["fc2_w"], inp["fc2_b"], inp["ln3_g"], inp["ln3_b"],
        inp["fc3_w"], inp["fc3_b"])          # (L, heads)
    idx = _rel_index()                        # (N, N): rpb[h][n, m]
    rpb_full = p_mlp[idx]                     # (n, m, heads)
    # rpb[p, h, mc, n] = rpb_full[n, mc*128+p, h]  (transposed layout)
    rpb = np.ascontiguousarray(
        rpb_full.transpose(1, 0, 2).reshape(2, 128, N, NUM_HEADS)
        .transpose(1, 0, 2, 3)               # (p, mc, n, h)
        .transpose(0, 3, 1, 2))              # (p, h, mc, n)

    qkvb = inp["qkv_b"].reshape(1, 768)
    projb = inp["proj_b"].reshape(1, 256)
    has_qkvb = bool(np.any(qkvb))
    has_projb = bool(np.any(projb))

    shared = {
        "wqk": wqk, "wv": wv, "wp": wp, "rpb": rpb,
        "ident": np.eye(128, dtype=np.float32),
        "onesw": np.ones((128, 32), ml_dtypes.bfloat16),
        "zout": np.zeros((128, 3, 256), np.float32),
        "qkvb": qkvb, "projb": projb,
        "onesr": np.ones((1, 256), np.float32),
    }
    in_maps = [
        {"x": np.ascontiguousarray(x[c * BPC:(c + 1) * BPC]), **shared}
        for c in range(NCORES)
    ]
    return in_maps, has_qkvb, has_projb


def kernel(**inputs):
    inp = {k: np.asarray(v, dtype=np.float32) for k, v in inputs.items()}
    in_maps, has_qkvb, has_projb = host_prep(inp)
    nc = _build_program(has_qkvb, has_projb)
    res = run_bass_kernel_spmd(nc, in_maps, list(range(NCORES)))
    global _LAST_RESULTS
    _LAST_RESULTS = res.results
    out = np.concatenate([res.results[c]["out"] for c in range(NCORES)], axis=0)
    return out.reshape(-1, C, H, W).astype(np.float32)


if __name__ == "__main__":
    rng = np.random.default_rng(0)
    demo = {"x": rng.standard_normal((B, C, H, W), dtype=np.float32)}
    print("kernel module loaded")


# revision 13
# speedup vs baseline: 196.2957x; 196.2957x over previous
"""Trainium2 Bass kernel for DPB (dynamic-position-bias) windowed attention.

Full inputs in, full outputs out. Shards data-parallel over the batch/window
axis across 8 NeuronCores (32 windows per core); all weights and the position
bias table are replicated per core.

Per-core pipeline for each window b (N=256 tokens, C=256 ch, 8 heads, d=32):
  1. qkT = (Wqk x_b)          PE, fp32r, output (o, n) layout
  2. v   = (x_b^T Wv)         PE, output (m, o_v) layout
  3. per head pair: PSUM <- rpb bias (identity-matmul preload), then
     ST[m,n] += k_h^T q_h     K=32 row-packed matmuls (distinct PSUM banks)
  4. P = exp(ST)              ScalarE, FD=1024, PSUM->SBUF (no max-subtract:
                              logits are O(1) so fp32 exp is safe)
  5. out_h^T = v_h^T P, den_h = 1^T P   3-way col-packed matmuls
  6. outT = out_h^T * recip(den)        VectorE (softmax normalize)
  7. final = Wproj outT       PE, written back in (C, N) layout
"""

import os
import sys

sys.path.insert(0, "/opt/trn_rl_repo")

import ml_dtypes
import numpy as np
from contextlib import ExitStack

import concourse.bacc as bacc
import concourse.tile as tile
from concourse import mybir
from concourse.bass_utils import run_bass_kernel_spmd

F32 = mybir.dt.float32
F32R = mybir.dt.float32r
BF16 = mybir.dt.bfloat16
EXP = mybir.ActivationFunctionType.Exp

NUM_HEADS = 8
B, C, H, W = 256, 256, 16, 16
N = H * W            # 256 tokens per window
D = C // NUM_HEADS   # 32
PD = 16              # pos-MLP hidden dim
LN_EPS = 1e-5
NCORES = 8
BPC = int(os.environ.get("KERNEL_BPC", B // NCORES))  # windows per core (32)


# ---------------------------------------------------------------- host math
def _rel_index():
    coords = np.stack(np.meshgrid(np.arange(H), np.arange(W), indexing="ij"))
    cf = coords.reshape(2, -1)
    rel = (cf[:, :, None] - cf[:, None, :]).transpose(1, 2, 0)
    rel = rel + np.array([H - 1, W - 1])
    rel[..., 0] *= 2 * W - 1
    return rel.sum(-1)  # (N, N) int


def _biases():
    bh = np.arange(1 - H, H)
    bw = np.arange(1 - W, W)
    b = np.stack(np.meshgrid(bh, bw, indexing="ij")).reshape(2, -1).T
    return b.astype(np.float32)  # ((2H-1)(2W-1), 2)


def _ln(x, g, b):
    mu = x.mean(-1, keepdims=True)
    var = ((x - mu) ** 2).mean(-1, keepdims=True)
    return (x - mu) / np.sqrt(var + LN_EPS) * g + b


def _pos_mlp(pos_proj_w, pos_proj_b, ln1_g, ln1_b, fc1_w, fc1_b,
             ln2_g, ln2_b, fc2_w, fc2_b, ln3_g, ln3_b, fc3_w, fc3_b):
    p = _biases() @ pos_proj_w.T + pos_proj_b
    p = np.maximum(_ln(p, ln1_g, ln1_b), 0.0) @ fc1_w.T + fc1_b
    p = np.maximum(_ln(p, ln2_g, ln2_b), 0.0) @ fc2_w.T + fc2_b
    p = np.maximum(_ln(p, ln3_g, ln3_b), 0.0) @ fc3_w.T + fc3_b
    return p.astype(np.float32)  # (L, heads)


# ------------------------------------------------------------- bass program
_PROGRAM_CACHE = {}
_LAST_RESULTS = None


def _build_program(has_qkvb, has_projb):
    key = (has_qkvb, has_projb, os.environ.get("KERNEL_DEBUG", "0"),
           os.environ.get("KERNEL_REPS", "1"))
    if key in _PROGRAM_CACHE:
        return _PROGRAM_CACHE[key]

    nc = bacc.Bacc("TRN2", target_bir_lowering=False, debug=False)

    def din(name, shp, dt=F32R):
        return nc.dram_tensor(name, shp, dt, kind="ExternalInput").ap()

    x_d = din("x", [BPC, C, N])                 # per-core x slice, (b, c, n)
    wqk_d = din("wqk", [2, 128, 512])           # (c-chunk, c_p, o) q|k, q scaled
    wv_d = din("wv", [2, 128, 256])             # (c-chunk, c_p, o_v)
    wp_d = din("wp", [128, 3, 256])             # proj weights, permuted + padded
    rpb_d = din("rpb", [128, NUM_HEADS, 2, 256])  # transposed bias table
    ident_d = din("ident", [128, 128])
    onesw_d = din("onesw", [128, 32], BF16)
    zout_d = din("zout", [128, 3, 256])
    qkvb_d = din("qkvb", [1, 768])
    projb_d = din("projb", [1, 256])
    onesr_d = din("onesr", [1, 256])
    out_d = nc.dram_tensor("out", [BPC, C, N], F32, kind="ExternalOutput").ap()
    DEBUG = bool(int(os.environ.get("KERNEL_DEBUG", "0")))
    if DEBUG:
        dbg_qk = nc.dram_tensor("dbg_qk", [128, 4, 256], F32, kind="ExternalOutput").ap()
        dbg_v = nc.dram_tensor("dbg_v", [128, 2, 256], F32, kind="ExternalOutput").ap()
        dbg_p = nc.dram_tensor("dbg_p", [128, 2, 2, 256], F32, kind="ExternalOutput").ap()
        dbg_outT = nc.dram_tensor("dbg_outT", [128, 3, 256], F32, kind="ExternalOutput").ap()
        dbg_avden = nc.dram_tensor("dbg_avden", [128, 2, 256], F32, kind="ExternalOutput").ap()

    with tile.TileContext(nc) as tc, ExitStack() as ctx:
        consts = ctx.enter_context(tc.tile_pool(name="consts", bufs=1))
        px = ctx.enter_context(tc.tile_pool(name="px", bufs=4))
        pqk = ctx.enter_context(tc.tile_pool(name="pqk", bufs=4))
        pv = ctx.enter_context(tc.tile_pool(name="pv", bufs=4))
        pp = ctx.enter_context(tc.tile_pool(name="pp", bufs=6))
        prec = ctx.enter_context(tc.tile_pool(name="prec", bufs=3))
        pout = ctx.enter_context(tc.tile_pool(name="pout", bufs=4))
        # PSUM: st pair-tiles 2 banks x bufs=2 (4 banks) + shared 1-bank pool
        # bufs=4 (4 banks) = all 8 banks.
        pst = ctx.enter_context(tc.tile_pool(name="pst", bufs=2, space="PSUM"))
        psh = ctx.enter_context(tc.tile_pool(name="psh", bufs=4, space="PSUM"))

        wqk_s = consts.tile([128, 2, 512], F32R)
        nc.sync.dma_start(wqk_s[:], wqk_d.transpose([1, 0, 2]))
        wv_s = consts.tile([128, 2, 256], F32R)
        nc.sync.dma_start(wv_s[:], wv_d.transpose([1, 0, 2]))
        wp_s = consts.tile([128, 3, 256], F32R)
        nc.sync.dma_start(wp_s[:], wp_d[:])
        rpb_s = consts.tile([128, NUM_HEADS, 2, 256], F32R)
        nc.sync.dma_start(rpb_s[:], rpb_d[:])
        ident_s = consts.tile([128, 128], F32R)
        nc.sync.dma_start(ident_s[:], ident_d[:])
        onesw_s = consts.tile([128, 32], BF16)
        nc.sync.dma_start(onesw_s[:], onesw_d[:])
        qkvb_s = consts.tile([1, 768], F32R)
        nc.sync.dma_start(qkvb_s[:], qkvb_d[:])
        projb_s = consts.tile([1, 256], F32R)
        nc.sync.dma_start(projb_s[:], projb_d[:])
        onesr_s = consts.tile([1, 256], F32R)
        nc.sync.dma_start(onesr_s[:], onesr_d[:])

        # persistent outT buffers (alternating per batch parity); partitions
        # beyond the valid head rows stay zero so proj can use K=128 blindly
        outT = [consts.tile([128, 3, 256], F32R, name=f"outT{i}", tag=f"outT{i}")
                for i in range(2)]
        nc.sync.dma_start(outT[0][:], zout_d[:])
        nc.sync.dma_start(outT[1][:], zout_d[:])

        G3 = [(0, 1, 2), (3, 4, 5), (6, 7)]  # col-pack head groups

        reps = int(os.environ.get("KERNEL_REPS", "1"))
        rep_ctx = tc.For_i(0, reps, 1) if reps > 1 else None
        if rep_ctx is not None:
            rep_ctx.__enter__()
        for b in range(BPC):
            x_sb = px.tile([128, 2, 256], F32R, tag="x")
            nc.sync.dma_start(x_sb[:], x_d[b].rearrange("(cc p) n -> p cc n", p=128))

            # ---- qk projection: out (o, n), oc tiles 0..3 = q0 q1 k0 k1
            qk_ps = psh.tile([128, 2, 256], F32, tag="bank")
            qk_ps2 = psh.tile([128, 2, 256], F32, tag="bank")
            qk_psl = [qk_ps, qk_ps2]
            for oc in range(4):
                dst = qk_psl[oc // 2][:, oc % 2, :]
                osl = slice(oc * 128, oc * 128 + 128)
                nc.tensor.matmul(dst, wqk_s[:, 0, osl], x_sb[:, 0, :],
                                 start=True, stop=False)
                nc.tensor.matmul(dst, wqk_s[:, 1, osl], x_sb[:, 1, :],
                                 start=False, stop=not has_qkvb)
                if has_qkvb:
                    nc.tensor.matmul(dst, qkvb_s[:, osl], onesr_s[:, :],
                                     start=False, stop=True)
            qk_sb = pqk.tile([128, 4, 256], F32R, tag="qk")
            nc.vector.tensor_copy(qk_sb[:, 0:2, :], qk_ps[:, :, :])
            nc.vector.tensor_copy(qk_sb[:, 2:4, :], qk_ps2[:, :, :])
            if DEBUG and b == 0:
                nc.sync.dma_start(dbg_qk[:], qk_sb.bitcast(F32)[:])

            # ---- v projection: out (m, o_v)
            v_ps = psh.tile([128, 2, 256], F32, tag="bank")
            for mc in range(2):
                msl = slice(mc * 128, mc * 128 + 128)
                nc.tensor.matmul(v_ps[:, mc, :], x_sb[:, 0, msl], wv_s[:, 0, :],
                                 start=True, stop=False)
                nc.tensor.matmul(v_ps[:, mc, :], x_sb[:, 1, msl], wv_s[:, 1, :],
                                 start=False, stop=not has_qkvb)
                if has_qkvb:
                    nc.tensor.matmul(v_ps[:, mc, :], onesr_s[:, 0:128],
                                     qkvb_s[:, 512:768], start=False, stop=True)
            v_sb = pv.tile([128, 2, 256], BF16, tag="v")
            nc.vector.tensor_copy(v_sb[:], v_ps[:])
            if DEBUG and b == 0:
                dbgv_sb = pv.tile([128, 2, 256], F32, tag="dbgv")
                nc.vector.tensor_copy(dbgv_sb[:], v_sb[:])
                nc.sync.dma_start(dbg_v[:], dbgv_sb[:])

            # ---- attention scores + softmax numerator, head pairs
            p_tiles = []
            for pg in range(4):  # heads (2pg, 2pg+1)
                st = pst.tile([128, 2, 2, 256], F32, tag="st")
                for par in range(2):
                    h = 2 * pg + par
                    nc.tensor.matmul(st[:, par, :, :], ident_s[:, :],
                                     rpb_s[:, h, :, :], start=True, stop=False)
                for par in range(2):
                    h = 2 * pg + par
                    hl = h % 4
                    hsl = slice(hl * 32, hl * 32 + 32)
                    qoc = h // 4        # q tile col in qk_sb
                    koc = 2 + h // 4    # k tile col
                    for mc in range(2):
                        nc.tensor.matmul(
                            st[:, par, mc, :],
                            qk_sb[hsl, koc, mc * 128:mc * 128 + 128],
                            qk_sb[hsl, qoc, :],
                            start=False, stop=(mc == 1),
                            tile_position=(hl * 32, 0))
                p_sb = pp.tile([128, 2, 2, 256], BF16, tag="p")
                nc.scalar.activation(p_sb[:, :, :, :], st[:, :, :, :], EXP)
                p_tiles.append(p_sb)
                if DEBUG and b == 0 and pg == 0:
                    dbgp_sb = pp.tile([128, 2, 2, 256], F32, tag="dbgp")
                    nc.vector.tensor_copy(dbgp_sb[:], p_sb[:])
                    nc.sync.dma_start(dbg_p[:], dbgp_sb[:])

            # ---- attn @ v and denominators, 3-way col-packed groups
            for t, heads in enumerate(G3):
                # av and den interleave accumulation groups, so they must sit
                # in different PSUM banks (start=True clears whole-bank bits)
                av_ps = psh.tile([128, 256], F32, tag="bank")
                den_ps = psh.tile([128, 256], F32, tag="bank")
                for j, h in enumerate(heads):
                    jsl = slice(j * 32, j * 32 + 32)
                    for mc in range(2):
                        p_ap = p_tiles[h // 2][:, h % 2, mc, :]
                        nc.tensor.matmul(av_ps[jsl, :],
                                         v_sb[:, mc, h * 32:h * 32 + 32], p_ap,
                                         start=(mc == 0), stop=(mc == 1),
                                         tile_position=(0, j * 32))
                        nc.tensor.matmul(den_ps[jsl, :], onesw_s[:, :], p_ap,
                                         start=(mc == 0), stop=(mc == 1),
                                         tile_position=(0, j * 32))
                if DEBUG and b == 0 and t == 0:
                    dbgav_sb = prec.tile([128, 2, 256], F32, tag="dbgav")
                    nc.vector.tensor_copy(dbgav_sb[:, 0, :], av_ps[:])
                    nc.vector.tensor_copy(dbgav_sb[:, 1, :], den_ps[:])
                    nc.sync.dma_start(dbg_avden[:], dbgav_sb[:])
                nv = 32 * len(heads)
                rec = prec.tile([128, 256], F32, tag="rec")
                nc.vector.reciprocal(rec[0:nv, :], den_ps[0:nv, :])
                nc.vector.tensor_mul(outT[b % 2][0:nv, t, :],
                                     av_ps[0:nv, :], rec[0:nv, :])

            if DEBUG and b == 0:
                nc.sync.dma_start(dbg_outT[:], outT[0].bitcast(F32)[:])
            # ---- output projection: final (o, n)
            f_ps = psh.tile([128, 2, 256], F32, tag="bank")
            for oc in range(2):
                osl = slice(oc * 128, oc * 128 + 128)
                for t in range(3):
                    nc.tensor.matmul(f_ps[:, oc, :], wp_s[:, t, osl],
                                     outT[b % 2][:, t, :],
                                     start=(t == 0),
                                     stop=(t == 2 and not has_projb))
                if has_projb:
                    nc.tensor.matmul(f_ps[:, oc, :], projb_s[:, osl],
                                     onesr_s[:, :], start=False, stop=True)
            out_sb = pout.tile([128, 2, 256], F32, tag="out")
            nc.vector.tensor_copy(out_sb[:], f_ps[:])
            nc.sync.dma_start(out_d[b].rearrange("(oc p) n -> p oc n", p=128),
                              out_sb[:])
        if rep_ctx is not None:
            rep_ctx.__exit__(None, None, None)

    nc.compile()
    _PROGRAM_CACHE[key] = nc
    return nc


# ------------------------------------------------------------------ kernel
def host_prep(inp):
    """Returns (in_maps, has_qkvb, has_projb) for the 8 cores."""
    x = inp["x"].reshape(B, C, N)
    scale = float(D) ** -0.5

    # fold the attention scale into the q columns of qkv_w
    qkv_wT = inp["qkv_w"].T.copy()            # (c, o)
    qkv_wT[:, 0:C] *= scale
    wqk = np.ascontiguousarray(
        qkv_wT[:, 0:512].reshape(2, 128, 512))
    wv = np.ascontiguousarray(
        qkv_wT[:, 512:768].reshape(2, 128, 256))

    # proj weights: permuted into the 3-way col-pack outT layout, zero-padded
    proj_wT = inp["proj_w"].T.copy()          # (c, o)
    wp = np.zeros((128, 3, 256), np.float32)
    G3 = [(0, 1, 2), (3, 4, 5), (6, 7)]
    for t, heads in enumerate(G3):
        for j, h in enumerate(heads):
            wp[j * 32:(j + 1) * 32, t, :] = proj_wT[h * 32:(h + 1) * 32, :]

    # dynamic position bias table via the tiny MLP (host, fp32)
    p_mlp = _pos_mlp(
        inp["pos_proj_w"], inp["pos_proj_b"], inp["ln1_g"], inp["ln1_b"],
        inp["fc1_w"], inp["fc1_b"], inp["ln2_g"], inp["ln2_b"],
        inp["fc2_w"], inp["fc2_b"], inp["ln3_g"], inp["ln3_b"],
        inp["fc3_w"], inp["fc3_b"])          # (L, heads)
    idx = _rel_index()                        # (N, N): rpb[h][n, m]
    rpb_full = p_mlp[idx]                     # (n, m, heads)
    # rpb[p, h, mc, n] = rpb_full[n, mc*128+p, h]  (transposed layout)
    rpb = np.ascontiguousarray(
        rpb_full.transpose(1, 0, 2).reshape(2, 128, N, NUM_HEADS)
        .transpose(1, 0, 2, 3)               # (p, mc, n, h)
        .transpose(0, 3, 1, 2))              # (p, h, mc, n)

    qkvb = inp["qkv_b"].reshape(1, 768)
    projb = inp["proj_b"].reshape(1, 256)
    has_qkvb = bool(np.any(qkvb))
    has_projb = bool(np.any(projb))

    shared = {
        "wqk": wqk, "wv": wv, "wp": wp, "rpb": rpb,
        "ident": np.eye(128, dtype=np.float32),
        "onesw": np.ones((128, 32), ml_dtypes.bfloat16),
        "zout": np.zeros((128, 3, 256), np.float32),
        "qkvb": qkvb, "projb": projb,
        "onesr": np.ones((1, 256), np.float32),
    }
    in_maps = [
        {"x": np.ascontiguousarray(x[c * BPC:(c + 1) * BPC]), **shared}
        for c in range(NCORES)
    ]
    return in_maps, has_qkvb, has_projb


def kernel(**inputs):
    inp = {k: np.asarray(v, dtype=np.float32) for k, v in inputs.items()}
    in_maps, has_qkvb, has_projb = host_prep(inp)
    nc = _build_program(has_qkvb, has_projb)
    res = run_bass_kernel_spmd(nc, in_maps, list(range(NCORES)))
    global _LAST_RESULTS
    _LAST_RESULTS = res.results
    out = np.concatenate([res.results[c]["out"] for c in range(NCORES)], axis=0)
    return out.reshape(-1, C, H, W).astype(np.float32)


if __name__ == "__main__":
    rng = np.random.default_rng(0)
    demo = {"x": rng.standard_normal((B, C, H, W), dtype=np.float32)}
    print("kernel module loaded")


# revision 15
# speedup vs baseline: 198.8421x; 1.0130x over previous
"""Trainium2 Bass kernel for DPB (dynamic-position-bias) windowed attention.

Full inputs in, full outputs out. Shards data-parallel over the batch/window
axis across 8 NeuronCores (32 windows per core); all weights and the position
bias table are replicated per core.

Per-core pipeline for each window b (N=256 tokens, C=256 ch, 8 heads, d=32):
  1. qkT = (Wqk x_b)          PE, fp32r, output (o, n) layout
  2. v   = (x_b^T Wv)         PE, output (m, o_v) layout
  3. per head pair: PSUM <- rpb bias (identity-matmul preload), then
     ST[m,n] += k_h^T q_h     K=32 row-packed matmuls (distinct PSUM banks)
  4. P = exp(ST)              ScalarE, FD=1024, PSUM->SBUF (no max-subtract:
                              logits are O(1) so fp32 exp is safe)
  5. out_h^T = v_h^T P, den_h = 1^T P   3-way col-packed matmuls
  6. outT = out_h^T * recip(den)        VectorE (softmax normalize)
  7. final = Wproj outT       PE, written back in (C, N) layout
"""

import os
import sys

sys.path.insert(0, "/opt/trn_rl_repo")

import ml_dtypes
import numpy as np
from contextlib import ExitStack

import concourse.bacc as bacc
import concourse.tile as tile
from concourse import mybir
from concourse.bass_utils import run_bass_kernel_spmd

F32 = mybir.dt.float32
F32R = mybir.dt.float32r
BF16 = mybir.dt.bfloat16
EXP = mybir.ActivationFunctionType.Exp

NUM_HEADS = 8
B, C, H, W = 256, 256, 16, 16
N = H * W            # 256 tokens per window
D = C // NUM_HEADS   # 32
PD = 16              # pos-MLP hidden dim
LN_EPS = 1e-5
NCORES = 8
BPC = int(os.environ.get("KERNEL_BPC", B // NCORES))  # windows per core (32)


# ---------------------------------------------------------------- host math
def _rel_index():
    coords = np.stack(np.meshgrid(np.arange(H), np.arange(W), indexing="ij"))
    cf = coords.reshape(2, -1)
    rel = (cf[:, :, None] - cf[:, None, :]).transpose(1, 2, 0)
    rel = rel + np.array([H - 1, W - 1])
    rel[..., 0] *= 2 * W - 1
    return rel.sum(-1)  # (N, N) int


def _biases():
    bh = np.arange(1 - H, H)
    bw = np.arange(1 - W, W)
    b = np.stack(np.meshgrid(bh, bw, indexing="ij")).reshape(2, -1).T
    return b.astype(np.float32)  # ((2H-1)(2W-1), 2)


def _ln(x, g, b):
    mu = x.mean(-1, keepdims=True)
    var = ((x - mu) ** 2).mean(-1, keepdims=True)
    return (x - mu) / np.sqrt(var + LN_EPS) * g + b


def _pos_mlp(pos_proj_w, pos_proj_b, ln1_g, ln1_b, fc1_w, fc1_b,
             ln2_g, ln2_b, fc2_w, fc2_b, ln3_g, ln3_b, fc3_w, fc3_b):
    p = _biases() @ pos_proj_w.T + pos_proj_b
    p = np.maximum(_ln(p, ln1_g, ln1_b), 0.0) @ fc1_w.T + fc1_b
    p = np.maximum(_ln(p, ln2_g, ln2_b), 0.0) @ fc2_w.T + fc2_b
    p = np.maximum(_ln(p, ln3_g, ln3_b), 0.0) @ fc3_w.T + fc3_b
    return p.astype(np.float32)  # (L, heads)


# ------------------------------------------------------------- bass program
_PROGRAM_CACHE = {}
_LAST_RESULTS = None


def _build_program(has_qkvb, has_projb):
    key = (has_qkvb, has_projb, os.environ.get("KERNEL_DEBUG", "0"),
           os.environ.get("KERNEL_REPS", "1"))
    if key in _PROGRAM_CACHE:
        return _PROGRAM_CACHE[key]

    nc = bacc.Bacc("TRN2", target_bir_lowering=False, debug=False)

    def din(name, shp, dt=F32R):
        return nc.dram_tensor(name, shp, dt, kind="ExternalInput").ap()

    x_d = din("x", [BPC, C, N])                 # per-core x slice, (b, c, n)
    wqk_d = din("wqk", [2, 128, 512])           # (c-chunk, c_p, o) q|k, q scaled
    wv_d = din("wv", [2, 128, 256])             # (c-chunk, c_p, o_v)
    wp_d = din("wp", [128, 3, 256])             # proj weights, permuted + padded
    rpb_d = din("rpb", [128, NUM_HEADS, 2, 256])  # transposed bias table
    ident_d = din("ident", [128, 128])
    onesw_d = din("onesw", [128, 32], BF16)
    zout_d = din("zout", [128, 3, 2, 256])
    qkvb_d = din("qkvb", [1, 768])
    projb_d = din("projb", [1, 256])
    onesr_d = din("onesr", [1, 512])
    out_d = nc.dram_tensor("out", [BPC, C, N], F32, kind="ExternalOutput").ap()
    DEBUG = bool(int(os.environ.get("KERNEL_DEBUG", "0")))
    if DEBUG:
        dbg_qk = nc.dram_tensor("dbg_qk", [128, 4, 256], F32, kind="ExternalOutput").ap()
        dbg_v = nc.dram_tensor("dbg_v", [128, 2, 256], F32, kind="ExternalOutput").ap()
        dbg_p = nc.dram_tensor("dbg_p", [128, 2, 2, 256], F32, kind="ExternalOutput").ap()
        dbg_outT = nc.dram_tensor("dbg_outT", [128, 3, 256], F32, kind="ExternalOutput").ap()
        dbg_avden = nc.dram_tensor("dbg_avden", [128, 2, 256], F32, kind="ExternalOutput").ap()

    with tile.TileContext(nc) as tc, ExitStack() as ctx:
        consts = ctx.enter_context(tc.tile_pool(name="consts", bufs=1))
        px = ctx.enter_context(tc.tile_pool(name="px", bufs=4))
        pqk = ctx.enter_context(tc.tile_pool(name="pqk", bufs=4))
        pv = ctx.enter_context(tc.tile_pool(name="pv", bufs=4))
        pp = ctx.enter_context(tc.tile_pool(name="pp", bufs=6))
        prec = ctx.enter_context(tc.tile_pool(name="prec", bufs=3))
        pout = ctx.enter_context(tc.tile_pool(name="pout", bufs=4))
        # PSUM: st pair-tiles 2 banks x bufs=2 (4 banks) + shared 1-bank pool
        # bufs=4 (4 banks) = all 8 banks.
        pst = ctx.enter_context(tc.tile_pool(name="pst", bufs=2, space="PSUM"))
        psh = ctx.enter_context(tc.tile_pool(name="psh", bufs=4, space="PSUM"))

        wqk_s = consts.tile([128, 2, 512], F32R)
        nc.sync.dma_start(wqk_s[:], wqk_d.transpose([1, 0, 2]))
        wv_s = consts.tile([128, 2, 256], F32R)
        nc.sync.dma_start(wv_s[:], wv_d.transpose([1, 0, 2]))
        wp_s = consts.tile([128, 3, 256], F32R)
        nc.sync.dma_start(wp_s[:], wp_d[:])
        rpb_s = consts.tile([128, NUM_HEADS, 2, 256], F32R)
        nc.sync.dma_start(rpb_s[:], rpb_d[:])
        ident_s = consts.tile([128, 128], F32R)
        nc.sync.dma_start(ident_s[:], ident_d[:])
        onesw_s = consts.tile([128, 32], BF16)
        nc.sync.dma_start(onesw_s[:], onesw_d[:])
        qkvb_s = consts.tile([1, 768], F32R)
        nc.sync.dma_start(qkvb_s[:], qkvb_d[:])
        projb_s = consts.tile([1, 256], F32R)
        nc.sync.dma_start(projb_s[:], projb_d[:])
        onesr_s = consts.tile([1, 512], F32R)
        nc.sync.dma_start(onesr_s[:], onesr_d[:])

        # persistent outT buffers (alternating per batch parity); partitions
        # beyond the valid head rows stay zero so proj can use K=128 blindly
        outT = [consts.tile([128, 3, 2, 256], F32R, name=f"outT{i}", tag=f"outT{i}")
                for i in range(2)]
        nc.sync.dma_start(outT[0][:], zout_d[:])
        nc.sync.dma_start(outT[1][:], zout_d[:])

        G3 = [(0, 1, 2), (3, 4, 5), (6, 7)]  # col-pack head groups

        reps = int(os.environ.get("KERNEL_REPS", "1"))
        rep_ctx = tc.For_i(0, reps, 1) if reps > 1 else None
        if rep_ctx is not None:
            rep_ctx.__enter__()
        for bp in range(BPC // 2):
            x_sb = px.tile([128, 2, 2, 256], F32R, tag="x")
            for b01 in range(2):
                nc.sync.dma_start(
                    x_sb[:, :, b01, :],
                    x_d[2 * bp + b01].rearrange("(cc p) n -> p cc n", p=128))

            # ---- qk projection, both windows at FD=512: out (o, [b0|b1] n)
            qk_psl = [psh.tile([128, 512], F32, tag="bank", name=f"qkp{oc}")
                      for oc in range(4)]
            for oc in range(4):
                osl = slice(oc * 128, oc * 128 + 128)
                nc.tensor.matmul(qk_psl[oc][:], wqk_s[:, 0, osl],
                                 x_sb[:, 0, :, :], start=True, stop=False)
                nc.tensor.matmul(qk_psl[oc][:], wqk_s[:, 1, osl],
                                 x_sb[:, 1, :, :], start=False, stop=not has_qkvb)
                if has_qkvb:
                    nc.tensor.matmul(qk_psl[oc][:], qkvb_s[:, osl], onesr_s[:, :],
                                     start=False, stop=True)
            qk_sb = pqk.tile([128, 4, 2, 256], F32R, tag="qk")
            for oc in range(4):
                nc.vector.tensor_copy(qk_sb[:, oc, :, :], qk_psl[oc][:])
            if DEBUG and bp == 0:
                nc.sync.dma_start(dbg_qk[:], qk_sb.bitcast(F32)[:, :, 0, :])

            for b01 in range(2):
                b = 2 * bp + b01

                # ---- v projection: out (m, o_v), per window
                v_ps = psh.tile([128, 2, 256], F32, tag="bank")
                for mc in range(2):
                    msl = slice(mc * 128, mc * 128 + 128)
                    nc.tensor.matmul(v_ps[:, mc, :], x_sb[:, 0, b01, msl],
                                     wv_s[:, 0, :], start=True, stop=False)
                    nc.tensor.matmul(v_ps[:, mc, :], x_sb[:, 1, b01, msl],
                                     wv_s[:, 1, :], start=False, stop=not has_qkvb)
                    if has_qkvb:
                        nc.tensor.matmul(v_ps[:, mc, :], onesr_s[:, 0:128],
                                         qkvb_s[:, 512:768], start=False, stop=True)
                v_sb = pv.tile([128, 2, 256], BF16, tag="v")
                nc.vector.tensor_copy(v_sb[:], v_ps[:])
                if DEBUG and b == 0:
                    dbgv_sb = pv.tile([128, 2, 256], F32, tag="dbgv")
                    nc.vector.tensor_copy(dbgv_sb[:], v_sb[:])
                    nc.sync.dma_start(dbg_v[:], dbgv_sb[:])

                # ---- attention scores + softmax numerator, head pairs
                p_tiles = []
                for pg in range(4):  # heads (2pg, 2pg+1)
                    st = pst.tile([128, 2, 2, 256], F32, tag="st")
                    for par in range(2):
                        h = 2 * pg + par
                        nc.tensor.matmul(st[:, par, :, :], ident_s[:, :],
                                         rpb_s[:, h, :, :], start=True, stop=False)
                    for par in range(2):
                        h = 2 * pg + par
                        hl = h % 4
                        hsl = slice(hl * 32, hl * 32 + 32)
                        qoc = h // 4
                        koc = 2 + h // 4
                        for mc in range(2):
                            nc.tensor.matmul(
                                st[:, par, mc, :],
                                qk_sb[hsl, koc, b01, mc * 128:mc * 128 + 128],
                                qk_sb[hsl, qoc, b01, :],
                                start=False, stop=(mc == 1),
                                tile_position=(hl * 32, 0))
                    p_sb = pp.tile([128, 2, 2, 256], BF16, tag="p")
                    nc.scalar.activation(p_sb[:, :, :, :], st[:, :, :, :], EXP)
                    p_tiles.append(p_sb)
                    if DEBUG and b == 0 and pg == 0:
                        dbgp_sb = pp.tile([128, 2, 2, 256], F32, tag="dbgp")
                        nc.vector.tensor_copy(dbgp_sb[:], p_sb[:])
                        nc.sync.dma_start(dbg_p[:], dbgp_sb[:])

                # ---- attn @ v and denominators, 3-way col-packed groups
                for t, heads in enumerate(G3):
                    av_ps = psh.tile([128, 256], F32, tag="bank")
                    den_ps = psh.tile([128, 256], F32, tag="bank")
                    for j, h in enumerate(heads):
                        jsl = slice(j * 32, j * 32 + 32)
                        for mc in range(2):
                            p_ap = p_tiles[h // 2][:, h % 2, mc, :]
                            nc.tensor.matmul(av_ps[jsl, :],
                                             v_sb[:, mc, h * 32:h * 32 + 32], p_ap,
                                             start=(mc == 0), stop=(mc == 1),
                                             tile_position=(0, j * 32))
                            nc.tensor.matmul(den_ps[jsl, :], onesw_s[:, :], p_ap,
                                             start=(mc == 0), stop=(mc == 1),
                                             tile_position=(0, j * 32))
                    if DEBUG and b == 0 and t == 0:
                        dbgav_sb = prec.tile([128, 2, 256], F32, tag="dbgav")
                        nc.vector.tensor_copy(dbgav_sb[:, 0, :], av_ps[:])
                        nc.vector.tensor_copy(dbgav_sb[:, 1, :], den_ps[:])
                        nc.sync.dma_start(dbg_avden[:], dbgav_sb[:])
                    nv = 32 * len(heads)
                    rec = prec.tile([128, 256], F32, tag="rec")
                    nc.vector.reciprocal(rec[0:nv, :], den_ps[0:nv, :])
                    nc.vector.tensor_mul(outT[bp % 2][0:nv, t, b01, :],
                                         av_ps[0:nv, :], rec[0:nv, :])
                if DEBUG and b == 0:
                    nc.sync.dma_start(dbg_outT[:], outT[0].bitcast(F32)[:, :, 0, :])

            # ---- output projection, both windows at FD=512
            f_psl = [psh.tile([128, 512], F32, tag="bank", name=f"fp{oc}")
                     for oc in range(2)]
            for oc in range(2):
                osl = slice(oc * 128, oc * 128 + 128)
                for t in range(3):
                    nc.tensor.matmul(f_psl[oc][:], wp_s[:, t, osl],
                                     outT[bp % 2][:, t, :, :],
                                     start=(t == 0),
                                     stop=(t == 2 and not has_projb))
                if has_projb:
                    nc.tensor.matmul(f_psl[oc][:], projb_s[:, osl],
                                     onesr_s[:, :], start=False, stop=True)
            out_sb = pout.tile([128, 2, 2, 256], F32, tag="out")
            for oc in range(2):
                nc.vector.tensor_copy(out_sb[:, oc, :, :], f_psl[oc][:])
            for b01 in range(2):
                nc.sync.dma_start(
                    out_d[2 * bp + b01].rearrange("(oc p) n -> p oc n", p=128),
                    out_sb[:, :, b01, :])
        if rep_ctx is not None:
            rep_ctx.__exit__(None, None, None)

    nc.compile()
    _PROGRAM_CACHE[key] = nc
    return nc


# ------------------------------------------------------------------ kernel
def host_prep(inp):
    """Returns (in_maps, has_qkvb, has_projb) for the 8 cores."""
    x = inp["x"].reshape(B, C, N)
    scale = float(D) ** -0.5

    # fold the attention scale into the q columns of qkv_w
    qkv_wT = inp["qkv_w"].T.copy()            # (c, o)
    qkv_wT[:, 0:C] *= scale
    wqk = np.ascontiguousarray(
        qkv_wT[:, 0:512].reshape(2, 128, 512))
    wv = np.ascontiguousarray(
        qkv_wT[:, 512:768].reshape(2, 128, 256))

    # proj weights: permuted into the 3-way col-pack outT layout, zero-padded
    proj_wT = inp["proj_w"].T.copy()          # (c, o)
    wp = np.zeros((128, 3, 256), np.float32)
    G3 = [(0, 1, 2), (3, 4, 5), (6, 7)]
    for t, heads in enumerate(G3):
        for j, h in enumerate(heads):
            wp[j * 32:(j + 1) * 32, t, :] = proj_wT[h * 32:(h + 1) * 32, :]

    # dynamic position bias table via the tiny MLP (host, fp32)
    p_mlp = _pos_mlp(
        inp["pos_proj_w"], inp["pos_proj_b"], inp["ln1_g"], inp["ln1_b"],
        inp["fc1_w"], inp["fc1_b"], inp["ln2_g"], inp["ln2_b"],
        inp["fc2_w"], inp["fc2_b"], inp["ln3_g"], inp["ln3_b"],
        inp["fc3_w"], inp["fc3_b"])          # (L, heads)
    idx = _rel_index()                        # (N, N): rpb[h][n, m]
    rpb_full = p_mlp[idx]                     # (n, m, heads)
    # rpb[p, h, mc, n] = rpb_full[n, mc*128+p, h]  (transposed layout)
    rpb = np.ascontiguousarray(
        rpb_full.transpose(1, 0, 2).reshape(2, 128, N, NUM_HEADS)
        .transpose(1, 0, 2, 3)               # (p, mc, n, h)
        .transpose(0, 3, 1, 2))              # (p, h, mc, n)

    qkvb = inp["qkv_b"].reshape(1, 768)
    projb = inp["proj_b"].reshape(1, 256)
    has_qkvb = bool(np.any(qkvb))
    has_projb = bool(np.any(projb))

    shared = {
        "wqk": wqk, "wv": wv, "wp": wp, "rpb": rpb,
        "ident": np.eye(128, dtype=np.float32),
        "onesw": np.ones((128, 32), ml_dtypes.bfloat16),
        "zout": np.zeros((128, 3, 2, 256), np.float32),
        "qkvb": qkvb, "projb": projb,
        "onesr": np.ones((1, 512), np.float32),
    }
    in_maps = [
        {"x": np.ascontiguousarray(x[c * BPC:(c + 1) * BPC]), **shared}
        for c in range(NCORES)
    ]
    return in_maps, has_qkvb, has_projb


def kernel(**inputs):
    inp = {k: np.asarray(v, dtype=np.float32) for k, v in inputs.items()}
    in_maps, has_qkvb, has_projb = host_prep(inp)
    nc = _build_program(has_qkvb, has_projb)
    res = run_bass_kernel_spmd(nc, in_maps, list(range(NCORES)))
    global _LAST_RESULTS
    _LAST_RESULTS = res.results
    out = np.concatenate([res.results[c]["out"] for c in range(NCORES)], axis=0)
    return out.reshape(-1, C, H, W).astype(np.float32)


if __name__ == "__main__":
    rng = np.random.default_rng(0)
    demo = {"x": rng.standard_normal((B, C, H, W), dtype=np.float32)}
    print("kernel module loaded")
